# revision 28
# baseline (speedup 1.0000x reference)
"""RBF-kernel attention (dense_transformer) on 8 TRN2 NeuronCores.

Reference computation (B=1, S=4096, D=768, H=12, Dh=64):
    q,k,v = x@Wq, x@Wk, x@Wv               (per-head split)
    dist  = ||q_s - k_t||^2
    scores= exp(-gamma_h/8 * dist)
    out   = (scores @ v) merged @ Wo

Sharding: 8-way data parallel over query rows (512 rows/core).  Each core
computes its local K/V shard + per-head k-norms, all-gathers an augmented
K (rows: [k(64); kn_hi; kn_lo; 1; 1; 0-pad]) and V across cores, then
computes the full distance matrix for its queries with a single 80-deep
matmul per tile:
    dist[t,s] = kaug[:,t] . qaug[:,s],  qaug = [-2q; 1; 1; qn_hi; qn_lo; 0]
(contraction padded to 80: K%16 != 0 streams at half rate on the PE).

The kernel is ACT(exp)-bound: 12 heads x 4096 keys x 512 queries = 25.2M
exps per core at ~1 elem/lane/cycle @1.2GHz ~= 170us.  Everything else is
arranged to hide under that:
  - gathers split into 4 sub-1MB pieces ordered so the h0/h1 kaug part
    has no V dependency and fires ~10us in; phase B streams per-head as
    parts land.
  - K dt=0 projection -> gather A -> Q dt=0 run first so the first dist
    can issue as soon as part A lands.
  - attn@V runs as column-tiled head PAIRS (M=64 each, col groups 0-1 /
    2-3) into one packed [128,512] PSUM tile - both heads' AV matmuls
    execute concurrently on the PE.
  - big fused DMAs (3D access patterns): one per weight dt-block, one per
    gathered head (kaug [80, 8x512]), one per core for gathered V.
exp runs on the scalar engine straight out of PSUM with the per-head
scale folded in, over 1536-column groups to amortize ACT overhead.
attn@V is computed transposed (out^T[d,s]) so no on-chip transposes are
needed, and the final Wo matmul emits the core's output slice transposed
([768, 512]); the host transposes and concatenates.  All TensorE-facing
data is bf16 (fp32 PSUM accumulation); k/q norms get a hi+lo bf16 split
so the exponent stays fp32-accurate.
"""

import numpy as np
import ml_dtypes

N_CORES = 8
S = 4096          # sequence length
D = 768           # embed dim
H = 12            # heads
DH = 64           # head dim
SL = S // N_CORES # query rows per core (512)
P = 128
KC = D // P       # contraction chunks for projections (6)
NAUG = DH + 4     # meaningful aug rows (68)
AUG = 80          # padded to mult-of-16: K%16!=0 matmuls stream at half rate
SCALE = 1.0 / np.sqrt(DH)
DHALF = D // 2    # V gathered in two column halves

_BF16 = ml_dtypes.bfloat16


def build(neg_a):
    """Build the SPMD Bass graph. neg_a: list of 12 floats (-gamma[h]*SCALE)."""
    import concourse.bass as bass  # noqa: F401
    import concourse.mybir as mybir
    import concourse.tile as tile
    from concourse import bacc

    fb = mybir.dt.bfloat16
    f32 = mybir.dt.float32

    nc = bacc.Bacc("TRN2", target_bir_lowering=False, debug=False,
                   num_devices=N_CORES)

    xT = nc.dram_tensor("xT", [D, SL], fb, kind="ExternalInput").ap()
    wq = nc.dram_tensor("wq", [D, D], fb, kind="ExternalInput").ap()
    wk = nc.dram_tensor("wk", [D, D], fb, kind="ExternalInput").ap()
    wv = nc.dram_tensor("wv", [D, D], fb, kind="ExternalInput").ap()
    wo = nc.dram_tensor("wo", [D, D], fb, kind="ExternalInput").ap()
    outT = nc.dram_tensor("outT", [D, SL], f32, kind="ExternalOutput").ap()

    # four fused sub-1MB gathers (>1MB would pick hang-prone RDH).  The CC
    # engine can't begin its first ALGO_MESH until ~75us in (fixed startup
    # + core skew), then processes parts serially (~25us each), so parts
    # are ordered by phase-B need time:
    #   part0 = kaug h0,h1                  (first dist/EXP)
    #   part1 = V cols 0:384 + kaug h2,h3   (first attn@V)
    #   part2 = kaug h4,h5 + V cols 384:768
    #   part3 = kaug h6..h11
    A_SZ = AUG * SL
    V_SZ = SL * DHALF
    PART_SIZES = [2 * A_SZ, V_SZ + 2 * A_SZ, 2 * A_SZ + V_SZ, 6 * A_SZ]
    HOFF = {0: (0, 0), 1: (0, A_SZ),
            2: (1, V_SZ), 3: (1, V_SZ + A_SZ),
            4: (2, 0), 5: (2, A_SZ),
            6: (3, 0), 7: (3, A_SZ), 8: (3, 2 * A_SZ), 9: (3, 3 * A_SZ),
            10: (3, 4 * A_SZ), 11: (3, 5 * A_SZ)}
    VOFFS = {"a": (1, 0), "b": (2, 2 * A_SZ)}
    fsend = [nc.dram_tensor(f"fsend{p}", [PART_SIZES[p]], fb)
             for p in range(4)]
    fg = [nc.dram_tensor(f"fg{p}", [N_CORES * PART_SIZES[p]], fb,
                         addr_space="Shared") for p in range(4)]
    rg = [list(range(N_CORES))]

    def ksend2d(h, row, nrows):
        p, off = HOFF[h]
        base = off + row * SL
        return fsend[p][base:base + nrows * SL].rearrange("(a b) -> a b", b=SL)

    def vsend2d(which, trow, nrows):
        p, off = VOFFS[which]
        base = off + trow * DHALF
        return fsend[p][base:base + nrows * DHALF].rearrange(
            "(a b) -> a b", b=DHALF)

    def kg3d(h):
        """Gathered kaug for head h across all cores: [AUG, 8, SL] src AP."""
        p, off = HOFF[h]
        ps = PART_SIZES[p]
        return fg[p].rearrange("(c x) -> c x", x=ps)[
            :, off:off + A_SZ].rearrange("c (r q) -> r c q", q=SL)

    def vg3d(which, c):
        """Gathered V half of core c: [128, 4, DHALF] src AP (rows t)."""
        p, off = VOFFS[which]
        ps = PART_SIZES[p]
        return fg[p].rearrange("(c x) -> c x", x=ps)[
            c, off:off + V_SZ].rearrange("(b t q) -> t b q", t=P, q=DHALF)

    with tile.TileContext(nc) as tc:
        with tc.tile_pool(name="persist", bufs=1) as pp:
            # x^T resident: [128, 6*512] (k-chunk major)
            xT_sb = pp.tile([P, KC * SL], fb, name="xT_sb")
            wo_sb = pp.tile([P, KC * D], fb, name="wo_sb")
            qaug = [pp.tile([AUG, SL], fb, name=f"qaug{h}") for h in range(H)]
            # gathered V, one tile per source core: cols = (j%4)*384 + h*64
            vga_sb = [pp.tile([P, 4 * DHALF], fb, name=f"vga_sb{c}")
                      for c in range(N_CORES)]
            vgb_sb = [pp.tile([P, 4 * DHALF], fb, name=f"vgb_sb{c}")
                      for c in range(N_CORES)]
            ot_sb = [pp.tile([P, SL], fb, name=f"ot_sb{m}") for m in range(KC)]
            hsel = pp.tile([P, 2], f32, name="hsel")

            # x^T first: gates the dt=0 projections (separate DMAs so they
            # spread across DMA engines).
            for k in range(KC):
                nc.scalar.dma_start(xT_sb[:, k * SL:(k + 1) * SL],
                                    xT[k * P:(k + 1) * P, :])

            # head-pair selector for partition-sum via matmul:
            # col j sums partitions j*64..j*64+63
            nc.vector.memset(hsel[:], 0.0)
            nc.vector.memset(hsel[0:DH, 0:1], 1.0)
            nc.vector.memset(hsel[DH:P, 1:2], 1.0)

            # [1,1,0,...]: K-side aug rows 66..80 in one DMA per head
            onz_sb = pp.tile([AUG - NAUG + 2, SL], fb, name="onz_sb")
            nc.vector.memset(onz_sb[:], 0.0)
            nc.vector.memset(onz_sb[0:2, :], 1.0)

            # qaug constant rows set once: zero pad (rows 68:80 stay zero
            # forever) and the two ones rows (64:66, 32-aligned offset).
            for h in range(H):
                nc.vector.memset(qaug[h][:], 0.0)
                nc.vector.memset(qaug[h][DH:DH + 2, :], 1.0)

            # ---------------- phase A: projections + aug build -------------
            with tc.tile_pool(name="psA", bufs=3, space="PSUM") as psA, \
                 tc.tile_pool(name="psN", bufs=2, space="PSUM") as psN, \
                 tc.tile_pool(name="workA", bufs=3) as wa:

                wk_sb = wa.tile([P, KC * D], fb, name="wk_sb", bufs=1)
                wq_sb = wa.tile([P, KC * D], fb, name="wq_sb", bufs=1)
                wv_sb = wa.tile([P, KC * D], fb, name="wv_sb", bufs=1)
                # per-dt column blocks so the dt=0 inputs land first
                for w_sb, w_dram in ((wk_sb, wk), (wq_sb, wq)):
                    for dt in range(KC):
                        nc.sync.dma_start(
                            w_sb[:].rearrange("p (k c) -> p k c", c=D)[
                                :, :, dt * P:(dt + 1) * P],
                            w_dram.rearrange("(k p) c -> p k c", p=P)[
                                :, :, dt * P:(dt + 1) * P])
                nc.sync.dma_start(
                    wv_sb[:].rearrange("p (k c) -> p k c", c=D),
                    wv.rearrange("(k p) c -> p k c", p=P))
                # constant kaug rows 66..80: h0/h1 first (they gate gather
                # A), rest later.  Sends live on the scalar queue, ahead of
                # the phase-B EXPs.
                for h in range(2):
                    nc.scalar.dma_start(ksend2d(h, DH + 2, AUG - DH - 2),
                                        onz_sb[:])

                def project_T(w_sb, dt):
                    """psum[128, SL] = (W^T x^T) rows dt*128..+128."""
                    ps = psA.tile([P, SL], f32, name=f"projT{dt}", tag="projT")
                    for k in range(KC):
                        nc.tensor.matmul(
                            ps[:], lhsT=w_sb[:, k * D + dt * P:k * D + (dt + 1) * P],
                            rhs=xT_sb[:, k * SL:(k + 1) * SL], start=(k == 0),
                            stop=(k == KC - 1))
                    return ps

                def norms(ps_bf, dt, tag):
                    """hi/lo bf16 split of per-head sum of squares.

                    Returns [34, SL] tile: rows 0:2 = hi (head pair), rows
                    32:34 = lo — 32-aligned so compute engines may write both,
                    and nhl[half::32] DMAs one head's (hi, lo) pair at once.
                    """
                    sq = wa.tile([P, SL], f32, name=f"sq_{tag}{dt}", tag="sq")
                    nc.vector.tensor_mul(sq[:], ps_bf[:], ps_bf[:])
                    nps = psN.tile([2, SL], f32, name=f"n_{tag}{dt}", tag="norm")
                    nc.tensor.matmul(nps[:], lhsT=hsel[:], rhs=sq[:],
                                     start=True, stop=True)
                    nhl = wa.tile([34, SL], fb, name=f"nhl_{tag}{dt}", tag="nhl")
                    nc.vector.tensor_copy(nhl[0:2, :], nps[:])
                    nc.vector.tensor_sub(nhl[32:34, :], nps[:], nhl[0:2, :])
                    return nhl

                def k_side(dt):
                    ps = project_T(wk_sb, dt)
                    ktb = wa.tile([P, SL], fb, name=f"ktb{dt}", tag="ktb")
                    nc.vector.tensor_copy(ktb[:], ps[:])
                    nhl = norms(ktb, dt, "k")
                    for half in range(2):
                        h = 2 * dt + half
                        nc.scalar.dma_start(ksend2d(h, 0, DH),
                                            ktb[half * DH:(half + 1) * DH, :])
                        nc.scalar.dma_start(ksend2d(h, DH, 2),
                                            nhl[half:34:32, :])
                        if h >= 2:
                            nc.scalar.dma_start(
                                ksend2d(h, DH + 2, AUG - DH - 2), onz_sb[:])

                def q_side(dt):
                    ps = project_T(wq_sb, dt)
                    qtb = wa.tile([P, SL], fb, name=f"qtb{dt}", tag="ktb")
                    nc.vector.tensor_copy(qtb[:], ps[:])
                    nhl = norms(qtb, dt, "q")
                    for half in range(2):
                        h = 2 * dt + half
                        qa = qaug[h]
                        nc.vector.tensor_scalar_mul(
                            qa[0:DH, :], qtb[half * DH:(half + 1) * DH, :], -2.0)
                        # rows 66:68 ([qn_hi; qn_lo]): partition offset 66
                        # isn't 32-aligned for compute engines -> DMA
                        nc.sync.dma_start(qa[DH + 2:DH + 4, :],
                                          nhl[half:34:32, :])

                def gather(p):
                    nc.gpsimd.collective_compute(
                        "AllGather", mybir.AluOpType.bypass,
                        ins=[fsend[p][:]], outs=[fg[p][:]],
                        replica_groups=rg)

                def v_local():
                    # V local (natural layout, column halves)
                    for tt in range(SL // P):
                        vloc = wa.tile([P, D], fb, name=f"vloc{tt}", tag="vloc")
                        for nh in range(2):
                            ps = psA.tile([P, 384], f32, name=f"vps{tt}_{nh}",
                                          tag="vps")
                            for k in range(KC):
                                nc.tensor.matmul(
                                    ps[:],
                                    lhsT=xT_sb[:, k * SL + tt * P:
                                               k * SL + (tt + 1) * P],
                                    rhs=wv_sb[:, k * D + nh * 384:
                                              k * D + (nh + 1) * 384],
                                    start=(k == 0), stop=(k == KC - 1))
                            nc.vector.tensor_copy(
                                vloc[:, nh * 384:(nh + 1) * 384], ps[:])
                        nc.scalar.dma_start(vsend2d("a", tt * P, P),
                                            vloc[:, :DHALF])
                        nc.scalar.dma_start(vsend2d("b", tt * P, P),
                                            vloc[:, DHALF:])

                # ordering: K dt0 -> gather 0 -> Q dt0 (first dist inputs),
                # then K dt1 + V -> gather 1 (Va+k23), K dt2 -> gather 2,
                # K dt3..5 -> gather 3, Q dt1..5 fill in behind.
                k_side(0)
                gather(0)
                q_side(0)
                k_side(1)
                v_local()
                gather(1)
                k_side(2)
                gather(2)
                for dt in range(3, KC):
                    k_side(dt)
                gather(3)
                for dt in range(1, KC):
                    q_side(dt)

                nc.sync.dma_start(
                    wo_sb[:].rearrange("p (k c) -> p k c", c=D),
                    wo.rearrange("(k p) c -> p k c", p=P))

                # gathered V -> SBUF (gpsimd queue: after the A/B/C/D
                # triggers so they can't be delayed by these waits)
                for c in range(N_CORES):
                    nc.gpsimd.dma_start(
                        vga_sb[c][:].rearrange("t (b q) -> t b q", q=DHALF),
                        vg3d("a", c))
                for c in range(N_CORES):
                    nc.gpsimd.dma_start(
                        vgb_sb[c][:].rearrange("t (b q) -> t b q", q=DHALF),
                        vg3d("b", c))

            # ---------------- phase B: scores + attn@V ---------------------
            # 3 t-chunks (1536 cols) per EXP to amortize ACT's 352-cycle
            # per-instruction overhead; PSUM: 2*3 (dist) + 2*1 (o_ps) = 8
            groups = [list(range(g * 3, min(32, g * 3 + 3)))
                      for g in range((32 + 2) // 3)]

            def vslice(j, h):
                if h < 6:
                    return vga_sb[j // 4][:, (j % 4) * DHALF + h * DH:
                                          (j % 4) * DHALF + (h + 1) * DH]
                return vgb_sb[j // 4][:, (j % 4) * DHALF + (h - 6) * DH:
                                      (j % 4) * DHALF + (h - 5) * DH]

            with tc.tile_pool(name="psD", bufs=2, space="PSUM") as psD, \
                 tc.tile_pool(name="psO", bufs=1, space="PSUM") as psO, \
                 tc.tile_pool(name="psF", bufs=1, space="PSUM") as psF, \
                 tc.tile_pool(name="workB", bufs=4) as wb:
                kgs = {}
                # HAM keep-warm: the PE must never idle ~1us+ per group
                # (ACT-bound steady state) or the clock gate re-throttles it
                # to 1.2GHz and the dist matmuls run 2x slow.  Dummy matmuls
                # into a scratch bank fill the gap; they have no consumers.
                fill_ps = psF.tile([P, SL], f32, name="fill_ps")

                def fillers(n):
                    for _ in range(n):
                        nc.tensor.matmul(fill_ps[:], lhsT=xT_sb[:, 0:P],
                                         rhs=xT_sb[:, 0:SL],
                                         start=True, stop=True)

                def kg_load(h):
                    # gathered head, one [80, 512] DMA per source core so
                    # the transfers spread across DMA engines; sync queue
                    # (idle in phase B).  h1 rides the scalar queue instead:
                    # it is idle until the first EXP (also part0-gated), so
                    # the 16 pair-0 pieces issue from two queues in parallel
                    # and the first dist groups don't starve.  Later odd
                    # heads must NOT do this: their semaphore waits would
                    # block earlier pairs' EXPs on the in-order queue.
                    kg = wb.tile([AUG, S], fb, name=f"kg{h}", tag="kg")
                    src = kg3d(h)
                    eng = nc.scalar if h == 1 else nc.sync
                    for c in range(N_CORES):
                        eng.dma_start(kg[:, c * SL:(c + 1) * SL],
                                      src[:, c, :])
                    kgs[h] = kg

                kg_load(0)
                kg_load(1)
                for hp in range(H // 2):
                    h0, h1 = 2 * hp, 2 * hp + 1
                    o_ps = psO.tile([P, SL], f32, name=f"o_ps{hp}", tag="o_ps")
                    for gi, grp in enumerate(groups):
                        if gi == 1 and hp < 5:
                            kg_load(2 * hp + 2)
                            kg_load(2 * hp + 3)
                        w = len(grp) * SL
                        dists = {}
                        scs = {}
                        for h in (h0, h1):
                            dist = psD.tile([P, 3 * SL], f32,
                                            name=f"dist{h}_{grp[0]}", tag="dist")
                            for idx, j in enumerate(grp):
                                nc.tensor.matmul(
                                    dist[:, idx * SL:(idx + 1) * SL],
                                    lhsT=kgs[h][:, j * P:(j + 1) * P],
                                    rhs=qaug[h][:], start=True, stop=True)
                            dists[h] = dist
                        for h in (h0, h1):
                            sc = wb.tile([P, 3 * SL], fb,
                                         name=f"sc{h}_{grp[0]}", tag="sc",
                                         bufs=16)
                            nc.scalar.activation(
                                sc[:, :w], dists[h][:, :w],
                                mybir.ActivationFunctionType.Exp,
                                scale=float(neg_a[h]))
                            scs[h] = sc
                        fillers(2)
                        # column-tiled pair: h0 -> col groups 0-1 (out rows
                        # 0:64), h1 -> col groups 2-3 (out rows 64:128); the
                        # two AV matmuls run concurrently on the PE.
                        for idx, j in enumerate(grp):
                            nc.tensor.matmul(
                                o_ps[0:DH, :], lhsT=vslice(j, h0),
                                rhs=scs[h0][:, idx * SL:(idx + 1) * SL],
                                start=(j == 0), stop=(j == 31))
                            nc.tensor.matmul(
                                o_ps[DH:P, :], lhsT=vslice(j, h1),
                                rhs=scs[h1][:, idx * SL:(idx + 1) * SL],
                                start=(j == 0), stop=(j == 31))
                    nc.vector.tensor_copy(ot_sb[hp][:], o_ps[:])

                # ------------- phase C: out^T = Wo^T @ O^T -----------------
                for nt in range(KC):
                    rps = psD.tile([P, 3 * SL], f32, name=f"rps{nt}", tag="dist")
                    for m in range(KC):
                        nc.tensor.matmul(
                            rps[:, :SL],
                            lhsT=wo_sb[:, m * D + nt * P:m * D + (nt + 1) * P],
                            rhs=ot_sb[m][:], start=(m == 0),
                            stop=(m == KC - 1))
                    rsb = wb.tile([P, SL], f32, name=f"rsb{nt}", tag="rsb",
                                  bufs=2)
                    nc.vector.tensor_copy(rsb[:], rps[:, :SL])
                    nc.gpsimd.dma_start(outT[nt * P:(nt + 1) * P, :], rsb[:])

    nc.compile()
    return nc


def prepare_in_maps(x, Wq, Wk, Wv, Wo):
    xT = np.ascontiguousarray(x.reshape(S, D).T)  # [768, 4096]
    wqb = Wq.astype(_BF16)
    wkb = Wk.astype(_BF16)
    wvb = Wv.astype(_BF16)
    wob = Wo.astype(_BF16)
    in_maps = []
    for c in range(N_CORES):
        in_maps.append({
            "xT": np.ascontiguousarray(xT[:, c * SL:(c + 1) * SL]).astype(_BF16),
            "wq": wqb, "wk": wkb, "wv": wvb, "wo": wob,
        })
    return in_maps


def postprocess(results):
    out = np.empty((S, D), np.float32)
    for c in range(N_CORES):
        out[c * SL:(c + 1) * SL, :] = results[c]["outT"].T
    return out.reshape(1, S, D)


_CACHE = {}


def _get_nc(gamma):
    key = tuple(np.asarray(gamma, np.float64).tolist())
    if key not in _CACHE:
        neg_a = [-float(g) * SCALE for g in gamma]
        _CACHE[key] = build(neg_a)
    return _CACHE[key]


def kernel(x, Wq, Wk, Wv, Wo, gamma):
    from concourse.bass_utils import run_bass_kernel_spmd

    x = np.asarray(x, np.float32)
    nc = _get_nc(np.asarray(gamma, np.float32))
    in_maps = prepare_in_maps(x, np.asarray(Wq, np.float32),
                              np.asarray(Wk, np.float32),
                              np.asarray(Wv, np.float32),
                              np.asarray(Wo, np.float32))
    res = run_bass_kernel_spmd(nc, in_maps, core_ids=list(range(N_CORES)))
    return postprocess(res.results)


# revision 29
# speedup vs baseline: 1.0150x; 1.0150x over previous
"""RBF-kernel attention (dense_transformer) on 8 TRN2 NeuronCores.

Reference computation (B=1, S=4096, D=768, H=12, Dh=64):
    q,k,v = x@Wq, x@Wk, x@Wv               (per-head split)
    dist  = ||q_s - k_t||^2
    scores= exp(-gamma_h/8 * dist)
    out   = (scores @ v) merged @ Wo

Sharding: 8-way data parallel over query rows (512 rows/core).  Each core
computes its local K/V shard + per-head k-norms, all-gathers an augmented
K (rows: [k(64); kn_hi; kn_lo; 1; 1; 0-pad]) and V across cores, then
computes the full distance matrix for its queries with a single 80-deep
matmul per tile:
    dist[t,s] = kaug[:,t] . qaug[:,s],  qaug = [-2q; 1; 1; qn_hi; qn_lo; 0]
(contraction padded to 80: K%16 != 0 streams at half rate on the PE).

The kernel is ACT(exp)-bound: 12 heads x 4096 keys x 512 queries = 25.2M
exps per core at ~1 elem/lane/cycle @1.2GHz ~= 170us.  Everything else is
arranged to hide under that:
  - gathers split into 4 sub-1MB pieces ordered so the h0/h1 kaug part
    has no V dependency and fires ~10us in; phase B streams per-head as
    parts land.
  - K dt=0 projection -> gather A -> Q dt=0 run first so the first dist
    can issue as soon as part A lands.
  - attn@V runs as column-tiled head PAIRS (M=64 each, col groups 0-1 /
    2-3) into one packed [128,512] PSUM tile - both heads' AV matmuls
    execute concurrently on the PE.
  - big fused DMAs (3D access patterns): one per weight dt-block, one per
    gathered head (kaug [80, 8x512]), one per core for gathered V.
exp runs on the scalar engine straight out of PSUM with the per-head
scale folded in, over 1536-column groups to amortize ACT overhead.
attn@V is computed transposed (out^T[d,s]) so no on-chip transposes are
needed, and the final Wo matmul emits the core's output slice transposed
([768, 512]); the host transposes and concatenates.  All TensorE-facing
data is bf16 (fp32 PSUM accumulation); k/q norms get a hi+lo bf16 split
so the exponent stays fp32-accurate.
"""

import numpy as np
import ml_dtypes

N_CORES = 8
S = 4096          # sequence length
D = 768           # embed dim
H = 12            # heads
DH = 64           # head dim
SL = S // N_CORES # query rows per core (512)
P = 128
KC = D // P       # contraction chunks for projections (6)
NAUG = DH + 4     # meaningful aug rows (68)
AUG = 80          # padded to mult-of-16: K%16!=0 matmuls stream at half rate
SCALE = 1.0 / np.sqrt(DH)
DHALF = D // 2    # V gathered in two column halves

_BF16 = ml_dtypes.bfloat16


def build(neg_a):
    """Build the SPMD Bass graph. neg_a: list of 12 floats (-gamma[h]*SCALE)."""
    import concourse.bass as bass  # noqa: F401
    import concourse.mybir as mybir
    import concourse.tile as tile
    from concourse import bacc

    fb = mybir.dt.bfloat16
    f32 = mybir.dt.float32

    nc = bacc.Bacc("TRN2", target_bir_lowering=False, debug=False,
                   num_devices=N_CORES)

    xT = nc.dram_tensor("xT", [D, SL], fb, kind="ExternalInput").ap()
    wq = nc.dram_tensor("wq", [D, D], fb, kind="ExternalInput").ap()
    wk = nc.dram_tensor("wk", [D, D], fb, kind="ExternalInput").ap()
    wv = nc.dram_tensor("wv", [D, D], fb, kind="ExternalInput").ap()
    wo = nc.dram_tensor("wo", [D, D], fb, kind="ExternalInput").ap()
    outT = nc.dram_tensor("outT", [D, SL], f32, kind="ExternalOutput").ap()

    # four fused sub-1MB gathers (>1MB would pick hang-prone RDH).  The CC
    # engine can't begin its first ALGO_MESH until ~75us in (fixed startup
    # + core skew), then processes parts serially (~25us each), so parts
    # are ordered by phase-B need time:
    #   part0 = kaug h0,h1                  (first dist/EXP)
    #   part1 = V cols 0:384 + kaug h2,h3   (first attn@V)
    #   part2 = kaug h4,h5 + V cols 384:768
    #   part3 = kaug h6..h11
    A_SZ = AUG * SL
    V_SZ = SL * DHALF
    PART_SIZES = [2 * A_SZ, V_SZ + 2 * A_SZ, 2 * A_SZ + V_SZ, 6 * A_SZ]
    HOFF = {0: (0, 0), 1: (0, A_SZ),
            2: (1, V_SZ), 3: (1, V_SZ + A_SZ),
            4: (2, 0), 5: (2, A_SZ),
            6: (3, 0), 7: (3, A_SZ), 8: (3, 2 * A_SZ), 9: (3, 3 * A_SZ),
            10: (3, 4 * A_SZ), 11: (3, 5 * A_SZ)}
    VOFFS = {"a": (1, 0), "b": (2, 2 * A_SZ)}
    fsend = [nc.dram_tensor(f"fsend{p}", [PART_SIZES[p]], fb)
             for p in range(4)]
    fg = [nc.dram_tensor(f"fg{p}", [N_CORES * PART_SIZES[p]], fb,
                         addr_space="Shared") for p in range(4)]
    rg = [list(range(N_CORES))]

    def ksend2d(h, row, nrows):
        p, off = HOFF[h]
        base = off + row * SL
        return fsend[p][base:base + nrows * SL].rearrange("(a b) -> a b", b=SL)

    def vsend2d(which, trow, nrows):
        p, off = VOFFS[which]
        base = off + trow * DHALF
        return fsend[p][base:base + nrows * DHALF].rearrange(
            "(a b) -> a b", b=DHALF)

    def kg3d(h):
        """Gathered kaug for head h across all cores: [AUG, 8, SL] src AP."""
        p, off = HOFF[h]
        ps = PART_SIZES[p]
        return fg[p].rearrange("(c x) -> c x", x=ps)[
            :, off:off + A_SZ].rearrange("c (r q) -> r c q", q=SL)

    def vg3d(which, c):
        """Gathered V half of core c: [128, 4, DHALF] src AP (rows t)."""
        p, off = VOFFS[which]
        ps = PART_SIZES[p]
        return fg[p].rearrange("(c x) -> c x", x=ps)[
            c, off:off + V_SZ].rearrange("(b t q) -> t b q", t=P, q=DHALF)

    with tile.TileContext(nc) as tc:
        with tc.tile_pool(name="persist", bufs=1) as pp:
            # x^T resident: [128, 6*512] (k-chunk major)
            xT_sb = pp.tile([P, KC * SL], fb, name="xT_sb")
            wo_sb = pp.tile([P, KC * D], fb, name="wo_sb")
            qaug = [pp.tile([AUG, SL], fb, name=f"qaug{h}") for h in range(H)]
            # gathered V, one tile per source core: cols = (j%4)*384 + h*64
            vga_sb = [pp.tile([P, 4 * DHALF], fb, name=f"vga_sb{c}")
                      for c in range(N_CORES)]
            vgb_sb = [pp.tile([P, 4 * DHALF], fb, name=f"vgb_sb{c}")
                      for c in range(N_CORES)]
            ot_sb = [pp.tile([P, SL], fb, name=f"ot_sb{m}") for m in range(KC)]
            hsel = pp.tile([P, 2], f32, name="hsel")

            # x^T first: gates the dt=0 projections (separate DMAs so they
            # spread across DMA engines).
            for k in range(KC):
                nc.scalar.dma_start(xT_sb[:, k * SL:(k + 1) * SL],
                                    xT[k * P:(k + 1) * P, :])

            # head-pair selector for partition-sum via matmul:
            # col j sums partitions j*64..j*64+63
            nc.vector.memset(hsel[:], 0.0)
            nc.vector.memset(hsel[0:DH, 0:1], 1.0)
            nc.vector.memset(hsel[DH:P, 1:2], 1.0)

            # [1,1,0,...]: K-side aug rows 66..80 in one DMA per head
            onz_sb = pp.tile([AUG - NAUG + 2, SL], fb, name="onz_sb")
            nc.vector.memset(onz_sb[:], 0.0)
            nc.vector.memset(onz_sb[0:2, :], 1.0)

            # qaug constant rows set once: zero pad (rows 68:80 stay zero
            # forever) and the two ones rows (64:66, 32-aligned offset).
            for h in range(H):
                nc.vector.memset(qaug[h][:], 0.0)
                nc.vector.memset(qaug[h][DH:DH + 2, :], 1.0)

            # ---------------- phase A: projections + aug build -------------
            with tc.tile_pool(name="psA", bufs=3, space="PSUM") as psA, \
                 tc.tile_pool(name="psN", bufs=2, space="PSUM") as psN, \
                 tc.tile_pool(name="workA", bufs=3) as wa:

                wk_sb = wa.tile([P, KC * D], fb, name="wk_sb", bufs=1)
                wq_sb = wa.tile([P, KC * D], fb, name="wq_sb", bufs=1)
                wv_sb = wa.tile([P, KC * D], fb, name="wv_sb", bufs=1)
                # per-dt column blocks so the dt=0 inputs land first
                for w_sb, w_dram in ((wk_sb, wk), (wq_sb, wq)):
                    for dt in range(KC):
                        nc.sync.dma_start(
                            w_sb[:].rearrange("p (k c) -> p k c", c=D)[
                                :, :, dt * P:(dt + 1) * P],
                            w_dram.rearrange("(k p) c -> p k c", p=P)[
                                :, :, dt * P:(dt + 1) * P])
                nc.sync.dma_start(
                    wv_sb[:].rearrange("p (k c) -> p k c", c=D),
                    wv.rearrange("(k p) c -> p k c", p=P))
                # constant kaug rows 66..80: h0/h1 first (they gate gather
                # A), rest later.  Sends live on the scalar queue, ahead of
                # the phase-B EXPs.
                for h in range(2):
                    nc.scalar.dma_start(ksend2d(h, DH + 2, AUG - DH - 2),
                                        onz_sb[:])

                def project_T(w_sb, dt):
                    """psum[128, SL] = (W^T x^T) rows dt*128..+128."""
                    ps = psA.tile([P, SL], f32, name=f"projT{dt}", tag="projT")
                    for k in range(KC):
                        nc.tensor.matmul(
                            ps[:], lhsT=w_sb[:, k * D + dt * P:k * D + (dt + 1) * P],
                            rhs=xT_sb[:, k * SL:(k + 1) * SL], start=(k == 0),
                            stop=(k == KC - 1))
                    return ps

                def norms(ps_bf, dt, tag):
                    """hi/lo bf16 split of per-head sum of squares.

                    Returns [34, SL] tile: rows 0:2 = hi (head pair), rows
                    32:34 = lo — 32-aligned so compute engines may write both,
                    and nhl[half::32] DMAs one head's (hi, lo) pair at once.
                    """
                    sq = wa.tile([P, SL], f32, name=f"sq_{tag}{dt}", tag="sq")
                    nc.vector.tensor_mul(sq[:], ps_bf[:], ps_bf[:])
                    nps = psN.tile([2, SL], f32, name=f"n_{tag}{dt}", tag="norm")
                    nc.tensor.matmul(nps[:], lhsT=hsel[:], rhs=sq[:],
                                     start=True, stop=True)
                    nhl = wa.tile([34, SL], fb, name=f"nhl_{tag}{dt}", tag="nhl")
                    nc.vector.tensor_copy(nhl[0:2, :], nps[:])
                    nc.vector.tensor_sub(nhl[32:34, :], nps[:], nhl[0:2, :])
                    return nhl

                def k_side(dt):
                    ps = project_T(wk_sb, dt)
                    ktb = wa.tile([P, SL], fb, name=f"ktb{dt}", tag="ktb")
                    nc.vector.tensor_copy(ktb[:], ps[:])
                    nhl = norms(ktb, dt, "k")
                    for half in range(2):
                        h = 2 * dt + half
                        nc.scalar.dma_start(ksend2d(h, 0, DH),
                                            ktb[half * DH:(half + 1) * DH, :])
                        nc.scalar.dma_start(ksend2d(h, DH, 2),
                                            nhl[half:34:32, :])
                        if h >= 2:
                            nc.scalar.dma_start(
                                ksend2d(h, DH + 2, AUG - DH - 2), onz_sb[:])

                def q_side(dt):
                    ps = project_T(wq_sb, dt)
                    qtb = wa.tile([P, SL], fb, name=f"qtb{dt}", tag="ktb")
                    nc.vector.tensor_copy(qtb[:], ps[:])
                    nhl = norms(qtb, dt, "q")
                    for half in range(2):
                        h = 2 * dt + half
                        qa = qaug[h]
                        nc.vector.tensor_scalar_mul(
                            qa[0:DH, :], qtb[half * DH:(half + 1) * DH, :], -2.0)
                        # rows 66:68 ([qn_hi; qn_lo]): partition offset 66
                        # isn't 32-aligned for compute engines -> DMA
                        nc.sync.dma_start(qa[DH + 2:DH + 4, :],
                                          nhl[half:34:32, :])

                def gather(p):
                    nc.gpsimd.collective_compute(
                        "AllGather", mybir.AluOpType.bypass,
                        ins=[fsend[p][:]], outs=[fg[p][:]],
                        replica_groups=rg)

                def v_local():
                    # V local (natural layout, column halves)
                    for tt in range(SL // P):
                        vloc = wa.tile([P, D], fb, name=f"vloc{tt}", tag="vloc")
                        for nh in range(2):
                            ps = psA.tile([P, 384], f32, name=f"vps{tt}_{nh}",
                                          tag="vps")
                            for k in range(KC):
                                nc.tensor.matmul(
                                    ps[:],
                                    lhsT=xT_sb[:, k * SL + tt * P:
                                               k * SL + (tt + 1) * P],
                                    rhs=wv_sb[:, k * D + nh * 384:
                                              k * D + (nh + 1) * 384],
                                    start=(k == 0), stop=(k == KC - 1))
                            nc.vector.tensor_copy(
                                vloc[:, nh * 384:(nh + 1) * 384], ps[:])
                        nc.scalar.dma_start(vsend2d("a", tt * P, P),
                                            vloc[:, :DHALF])
                        nc.scalar.dma_start(vsend2d("b", tt * P, P),
                                            vloc[:, DHALF:])

                # ordering: K dt0 -> gather 0 -> Q dt0 (first dist inputs),
                # then K dt1 + V -> gather 1 (Va+k23), K dt2 -> gather 2,
                # K dt3..5 -> gather 3, Q dt1..5 fill in behind.
                k_side(0)
                gather(0)
                q_side(0)
                k_side(1)
                v_local()
                gather(1)
                k_side(2)
                gather(2)
                for dt in range(3, KC):
                    k_side(dt)
                gather(3)
                for dt in range(1, KC):
                    q_side(dt)

                nc.sync.dma_start(
                    wo_sb[:].rearrange("p (k c) -> p k c", c=D),
                    wo.rearrange("(k p) c -> p k c", p=P))

                # gathered V -> SBUF (gpsimd queue: after the A/B/C/D
                # triggers so they can't be delayed by these waits)
                for c in range(N_CORES):
                    nc.gpsimd.dma_start(
                        vga_sb[c][:].rearrange("t (b q) -> t b q", q=DHALF),
                        vg3d("a", c))
                for c in range(N_CORES):
                    nc.gpsimd.dma_start(
                        vgb_sb[c][:].rearrange("t (b q) -> t b q", q=DHALF),
                        vg3d("b", c))

            # ---------------- phase B: scores + attn@V ---------------------
            # 3 t-chunks (1536 cols) per EXP to amortize ACT's 352-cycle
            # per-instruction overhead; PSUM: 2*3 (dist) + 2*1 (o_ps) = 8
            groups = [list(range(g * 3, min(32, g * 3 + 3)))
                      for g in range((32 + 2) // 3)]

            def vslice(j, h):
                if h < 6:
                    return vga_sb[j // 4][:, (j % 4) * DHALF + h * DH:
                                          (j % 4) * DHALF + (h + 1) * DH]
                return vgb_sb[j // 4][:, (j % 4) * DHALF + (h - 6) * DH:
                                      (j % 4) * DHALF + (h - 5) * DH]

            with tc.tile_pool(name="psD", bufs=2, space="PSUM") as psD, \
                 tc.tile_pool(name="psO", bufs=1, space="PSUM") as psO, \
                 tc.tile_pool(name="psF", bufs=1, space="PSUM") as psF, \
                 tc.tile_pool(name="workB", bufs=4) as wb:
                kgs = {}
                # HAM keep-warm: the PE must never idle ~1us+ per group
                # (ACT-bound steady state) or the clock gate re-throttles it
                # to 1.2GHz and the dist matmuls run 2x slow.  Dummy matmuls
                # into a scratch bank fill the gap; they have no consumers.
                fill_ps = psF.tile([P, SL], f32, name="fill_ps")

                def fillers(n):
                    for _ in range(n):
                        nc.tensor.matmul(fill_ps[:], lhsT=xT_sb[:, 0:P],
                                         rhs=xT_sb[:, 0:SL],
                                         start=True, stop=True)

                def kg_load(h):
                    # gathered head, one [80, 512] DMA per source core so
                    # the transfers spread across DMA engines; sync queue
                    # (idle in phase B).
                    kg = wb.tile([AUG, S], fb, name=f"kg{h}", tag="kg")
                    src = kg3d(h)
                    for c in range(N_CORES):
                        nc.sync.dma_start(kg[:, c * SL:(c + 1) * SL],
                                          src[:, c, :])
                    kgs[h] = kg

                kg_load(0)
                kg_load(1)
                for hp in range(H // 2):
                    h0, h1 = 2 * hp, 2 * hp + 1
                    o_ps = psO.tile([P, SL], f32, name=f"o_ps{hp}", tag="o_ps")
                    for gi, grp in enumerate(groups):
                        if gi == 1 and hp < 5:
                            kg_load(2 * hp + 2)
                            kg_load(2 * hp + 3)
                        w = len(grp) * SL
                        dists = {}
                        scs = {}
                        for h in (h0, h1):
                            dist = psD.tile([P, 3 * SL], f32,
                                            name=f"dist{h}_{grp[0]}", tag="dist")
                            for idx, j in enumerate(grp):
                                nc.tensor.matmul(
                                    dist[:, idx * SL:(idx + 1) * SL],
                                    lhsT=kgs[h][:, j * P:(j + 1) * P],
                                    rhs=qaug[h][:], start=True, stop=True)
                            dists[h] = dist
                        for h in (h0, h1):
                            sc = wb.tile([P, 3 * SL], fb,
                                         name=f"sc{h}_{grp[0]}", tag="sc",
                                         bufs=16)
                            nc.scalar.activation(
                                sc[:, :w], dists[h][:, :w],
                                mybir.ActivationFunctionType.Exp,
                                scale=float(neg_a[h]))
                            scs[h] = sc
                        fillers(2)
                        # column-tiled pair: h0 -> col groups 0-1 (out rows
                        # 0:64), h1 -> col groups 2-3 (out rows 64:128); the
                        # two AV matmuls run concurrently on the PE.
                        for idx, j in enumerate(grp):
                            nc.tensor.matmul(
                                o_ps[0:DH, :], lhsT=vslice(j, h0),
                                rhs=scs[h0][:, idx * SL:(idx + 1) * SL],
                                start=(j == 0), stop=(j == 31))
                            nc.tensor.matmul(
                                o_ps[DH:P, :], lhsT=vslice(j, h1),
                                rhs=scs[h1][:, idx * SL:(idx + 1) * SL],
                                start=(j == 0), stop=(j == 31))
                    nc.vector.tensor_copy(ot_sb[hp][:], o_ps[:])

                # ------------- phase C: out^T = Wo^T @ O^T -----------------
                for nt in range(KC):
                    rps = psD.tile([P, 3 * SL], f32, name=f"rps{nt}", tag="dist")
                    for m in range(KC):
                        nc.tensor.matmul(
                            rps[:, :SL],
                            lhsT=wo_sb[:, m * D + nt * P:m * D + (nt + 1) * P],
                            rhs=ot_sb[m][:], start=(m == 0),
                            stop=(m == KC - 1))
                    rsb = wb.tile([P, SL], f32, name=f"rsb{nt}", tag="rsb",
                                  bufs=2)
                    nc.vector.tensor_copy(rsb[:], rps[:, :SL])
                    nc.gpsimd.dma_start(outT[nt * P:(nt + 1) * P, :], rsb[:])

    nc.compile()
    return nc


def prepare_in_maps(x, Wq, Wk, Wv, Wo):
    xT = np.ascontiguousarray(x.reshape(S, D).T)  # [768, 4096]
    wqb = Wq.astype(_BF16)
    wkb = Wk.astype(_BF16)
    wvb = Wv.astype(_BF16)
    wob = Wo.astype(_BF16)
    in_maps = []
    for c in range(N_CORES):
        in_maps.append({
            "xT": np.ascontiguousarray(xT[:, c * SL:(c + 1) * SL]).astype(_BF16),
            "wq": wqb, "wk": wkb, "wv": wvb, "wo": wob,
        })
    return in_maps


def postprocess(results):
    out = np.empty((S, D), np.float32)
    for c in range(N_CORES):
        out[c * SL:(c + 1) * SL, :] = results[c]["outT"].T
    return out.reshape(1, S, D)


_CACHE = {}


def _get_nc(gamma):
    key = tuple(np.asarray(gamma, np.float64).tolist())
    if key not in _CACHE:
        neg_a = [-float(g) * SCALE for g in gamma]
        _CACHE[key] = build(neg_a)
    return _CACHE[key]


def kernel(x, Wq, Wk, Wv, Wo, gamma):
    from concourse.bass_utils import run_bass_kernel_spmd

    x = np.asarray(x, np.float32)
    nc = _get_nc(np.asarray(gamma, np.float32))
    in_maps = prepare_in_maps(x, np.asarray(Wq, np.float32),
                              np.asarray(Wk, np.float32),
                              np.asarray(Wv, np.float32),
                              np.asarray(Wo, np.float32))
    res = run_bass_kernel_spmd(nc, in_maps, core_ids=list(range(N_CORES)))
    return postprocess(res.results)
